# revision 52
# baseline (speedup 1.0000x reference)
"""Trainium2 Bass kernel for ConvMHSA (B=16, C=512, H=W=32, 8 heads).

Data-parallel over batch: each of the 8 NeuronCores processes 2 batches.

v2: fp8(e4m3) DoubleRow matmuls throughout the attention path.
  - QKV projection: x and W in fp8; kc-pair DoubleRow blocks -> 4x fewer
    PE cycles than bf16. q/k written fp8 by ScalarE (Identity+bias), v
    written fp8 by VectorE (add broadcast bias), strided into per-mt-pair
    vT tiles [128, 2, 8, 65] with a persistent ones column (PV emits the
    softmax denominators Z for free).
  - Scores: q,k fp8; stride-0 DoubleRow blocks double the 64-contraction
    product (2*k.q, folded into the exp scale) at 0.5 cyc/row; two heads
    packed on the PE via tile_position row groups.
  - exp: global shift K (softmax-invariant) keeps e in e4m3 range.
    Work is split across three engines: ScalarE native Exp -> fp8 out;
    VectorE and Pool compute Schraudolph exp directly into fp8 bits via
    one tensor_scalar (f32 -> uint8 saturating convert, bitcast e4m3).
  - PV: e and vT fp8, m-tile-pair DoubleRow; two n-chunks share one PSUM
    bank ([c0h0|c0h1|c1h0|c1h1] x 65); one reciprocal [128,4] and one
    strided tensor_tensor mul per group normalizes into bf16 attnT.
  - attnT -> attn via the DMA-engine xbar transpose (DRAM bounce),
    PE transposes + ScalarE copies in the drain.
  - Output projection stays bf16 (xbar transpose requires 2-byte dtype);
    residual+bias via scalar_tensor_tensor on Pool; y staged bf16.

Emission software-pipelines at head-pair granularity as in v1: stretch
(b, p) emits pair p's scores/exp interleaved with pair p-1's PV/norm
plus filler projection jobs.
"""

import os
import sys

sys.path.insert(0, "/opt/trn_rl_repo")

import numpy as np

B, C, H, W = 16, 512, 32, 32
HEADS = 8
HD = C // HEADS          # 64
N = H * W                # 1024
NCORES = 8
NB = B // NCORES         # batches per core = 2
KT = C // 128            # 4 contraction tiles of 128
MT = N // 128            # 8 m-tiles / n-chunks
NPAIR = HEADS // 2       # 4 head-pairs

KSHIFT = 2.25            # global exp shift (softmax-invariant)
A8 = 8.0 / np.log(2.0)   # schraudolph e4m3 scale
SCH_S = A8 * 0.0625      # applied to doubled scores
SCH_B = 56.0 - 0.4375 - A8 * KSHIFT

# exp engine shares (of 256 half-tiles): ScalarE / VectorE. (Pool/GPSIMD
# cannot access PSUM on TRN2, so it only does memsets.)
EXP_SHARES = {"A": 139, "D": 117}
EXP_UNITS = 256

_cache = {}


def _exp_order():
    acc = {k: 0.0 for k in EXP_SHARES}
    order = []
    for _ in range(EXP_UNITS):
        for k in acc:
            acc[k] += EXP_SHARES[k] / EXP_UNITS
        pick = max(acc, key=lambda k: acc[k])
        acc[pick] -= 1.0
        order.append(pick)
    return order


def _build_nc():
    import concourse.bass as bass
    import concourse.tile as tile
    import concourse.mybir as mybir
    from concourse import bacc

    F32 = mybir.dt.float32
    F32R = mybir.dt.float32r
    BF16 = mybir.dt.bfloat16
    F8E4 = mybir.dt.float8e4
    U8 = mybir.dt.uint8
    EXP = mybir.ActivationFunctionType.Exp
    COPY = mybir.ActivationFunctionType.Copy
    IDENT = mybir.ActivationFunctionType.Identity
    DR = mybir.MatmulPerfMode.DoubleRow
    MULT = mybir.AluOpType.mult
    ADD = mybir.AluOpType.add

    nc = bacc.Bacc("TRN2", target_bir_lowering=False, debug=False,
                   num_devices=NCORES)

    xs8 = nc.dram_tensor("xs8", (NB, 128, KT * N), F8E4, kind="ExternalInput").ap()
    xsb = nc.dram_tensor("xsb", (NB, 128, KT * N), BF16, kind="ExternalInput").ap()
    wqkvT = nc.dram_tensor("wqkvT", (128, KT * 3 * C), F8E4, kind="ExternalInput").ap()
    bqkv_col = nc.dram_tensor("bqkv_col", (128, 12), F32, kind="ExternalInput").ap()
    bqkv_row = nc.dram_tensor("bqkv_row", (1, 3 * C), F32R, kind="ExternalInput").ap()
    woTg = nc.dram_tensor("woTg", (C, C), BF16, kind="ExternalInput").ap()

    identD = nc.dram_tensor("identD", (128, 128), BF16, kind="ExternalInput").ap()
    y = nc.dram_tensor("y", (NB, C, N), BF16, kind="ExternalOutput").ap()

    exp_order = _exp_order()
    exp_i = [0]

    with tile.TileContext(nc) as tc:
        with tc.tile_pool(name="const", bufs=1) as const, \
             tc.tile_pool(name="xfp", bufs=2) as xfp, \
             tc.tile_pool(name="xbp", bufs=2) as xbp, \
             tc.tile_pool(name="qkp", bufs=1) as qkp, \
             tc.tile_pool(name="epool", bufs=20) as epool, \
             tc.tile_pool(name="vtp", bufs=2) as vtp, \
             tc.tile_pool(name="atp", bufs=3) as atp, \
             tc.tile_pool(name="afp", bufs=2) as afp, \
             tc.tile_pool(name="small", bufs=4) as small, \
             tc.tile_pool(name="rzp", bufs=8) as rzp, \
             tc.tile_pool(name="spool", bufs=4, space="PSUM") as spool, \
             tc.tile_pool(name="bank1", bufs=4, space="PSUM") as bank1, \
             tc.tile_pool(name="zdram", bufs=6, space="DRAM") as zdram:

            # ---- weights/x: big strided DMAs, first-needed columns first,
            # spread across the three HWDGE queues (SP/ACT/DVE idle here) ----
            wq_t = const.tile([128, KT * 3 * C], F8E4, tag="wq", name="wq_t")
            wq3 = wq_t.rearrange("p (k o) -> p k o", k=KT)
            wsrc = wqkvT.rearrange("p (k o) -> p k o", k=KT)
            # per-kc column layout: [q_p0 k_p0 q_p1 k_p1 ... | v]
            nc.scalar.dma_start(out=wq3[:, :, 0:256], in_=wsrc[:, :, 0:256])

            xf8_t0 = xfp.tile([128, KT * N], F8E4, tag="xf8", name="xf8_t")
            x83 = xf8_t0.rearrange("p (k n) -> p k n", k=KT)
            x0src = xs8[0].rearrange("p (k n) -> p k n", k=KT)
            nc.sync.dma_start(out=x83[:, :, 0:512], in_=x0src[:, :, 0:512])
            bq_col = const.tile([128, 12], F32, tag="bqcol")
            nc.scalar.dma_start(out=bq_col, in_=bqkv_col)
            nc.scalar.dma_start(out=x83[:, :, 512:N], in_=x0src[:, :, 512:N])
            # v columns, then the remaining q/k columns
            nc.sync.dma_start(out=wq3[:, :, 1024:1536], in_=wsrc[:, :, 1024:1536])
            ident = const.tile([128, 128], BF16, tag="ident", name="ident")
            nc.scalar.dma_start(out=ident, in_=identD)
            nc.scalar.dma_start(out=wq3[:, :, 256:1024], in_=wsrc[:, :, 256:1024])
            wo = []
            for kc in range(KT):
                t = const.tile([128, C], BF16, tag=f"wo{kc}", name=f"wo{kc}")
                nc.sync.dma_start(out=t, in_=woTg[128 * kc:128 * (kc + 1), :])
                wo.append(t)

            bv_bc = const.tile([128, C], F32, tag="bvbc")
            bv_src = bass.AP(tensor=bqkv_row.tensor, offset=2 * C,
                             ap=[[0, 128], [1, C]])
            nc.sync.dma_start(out=bv_bc, in_=bv_src.bitcast(F32))
            bkA = const.tile([128, 1], F32, tag="bkA")
            nc.gpsimd.memset(bkA, -KSHIFT)

            # bf16 x for the output-projection residual
            xb_t0 = xbp.tile([128, KT * N], BF16, tag="xb", name="xb_t")
            nc.sync.dma_start(out=xb_t0, in_=xsb[0])

            # PE p-state warm-up (results never read)
            warm = const.tile([128, 512], BF16, tag="warm", name="warm")
            nc.gpsimd.memset(warm, 1.0)
            for i in range(9):
                wps = bank1.tile([128, 512], F32, tag="bank1", name="wps")
                nc.tensor.matmul(wps, warm[:, 0:128], warm,
                                 start=True, stop=True)

            def load_x8(b):
                t = xfp.tile([128, KT * N], F8E4, tag="xf8", name="xf8_t")
                t3 = t.rearrange("p (k n) -> p k n", k=KT)
                src = xs8[b].rearrange("p (k n) -> p k n", k=KT)
                nc.sync.dma_start(out=t3[:, :, 0:512], in_=src[:, :, 0:512])
                nc.sync.dma_start(out=t3[:, :, 512:N], in_=src[:, :, 512:N])
                return t

            def load_xb(b):
                t = xbp.tile([128, KT * N], BF16, tag="xb", name="xb_t")
                nc.sync.dma_start(out=t, in_=xsb[b])
                return t

            def dr_ap(t, row0, nrows, col0, ncols, blk_stride):
                """[nrows, 2, ncols] AP with DoubleRow blocks at blk_stride."""
                return bass.AP(tensor=t.tensor,
                               offset=t.offset + row0 * t.ap[0][0] + col0,
                               ap=[[t.ap[0][0], nrows], [blk_stride, 2],
                                   [1, ncols]])

            # ---- building blocks ----
            def proj_qk_chunk(xf8, pair, which, nch, dest):
                """One (q|k, nch) 512-col chunk of the pair projection."""
                ot = pair if which == "q" else KT + pair
                col = 256 * pair + (0 if which == "q" else 128)
                ps = bank1.tile([128, 512], F32, tag="bank1", name="ps")
                for i in range(2):
                    for j in range(2):
                        nc.tensor.matmul(
                            ps[:, 256 * i:256 * (i + 1)],
                            dr_ap(wq_t, 0, 128, 1536 * 2 * j + col, 128, 1536),
                            dr_ap(xf8, 0, 128,
                                  1024 * 2 * j + 512 * nch + 256 * i, 256, 1024),
                            start=(j == 0), stop=(j == 1), perf_mode=DR)
                nc.scalar.activation(
                    out=dest[:, 512 * nch:512 * (nch + 1)], in_=ps,
                    func=IDENT, bias=bq_col[:, ot:ot + 1])

            def proj_vT_mtile(xf8, vt, half, mt):
                """v m-tile -> half of a vT pair tile [128, 2, 8, 65] fp8."""
                ps = bank1.tile([128, 512], F32, tag="bank1", name="ps")
                for i in range(2):
                    for j in range(2):
                        nc.tensor.matmul(
                            ps[:, 256 * i:256 * (i + 1)],
                            dr_ap(xf8, 0, 128, 1024 * 2 * j + 128 * mt, 128, 1024),
                            dr_ap(wq_t, 0, 128,
                                  1536 * 2 * j + 1024 + 256 * i, 256, 1536),
                            start=(j == 0), stop=(j == 1), perf_mode=DR)
                # v bias is folded into the residual on the host (softmax
                # weights sum to 1), so this is a plain convert-copy.
                dest = bass.AP(tensor=vt.tensor,
                               offset=vt.offset + 520 * half,
                               ap=[[vt.ap[0][0], 128], [65, HEADS], [1, HD]])
                nc.vector.tensor_add(
                    out=dest,
                    in0=ps.rearrange("p (a b) -> p a b", a=HEADS),
                    in1=bv_bc.rearrange("p (a b) -> p a b", a=HEADS))

            def scores_par(qt, kt_, mt, par):
                """Scores for one head of the pair at m-tile mt: two
                single-bank PSUM halves so the exp pipeline is 4 deep."""
                lo = 64 * par
                halves = []
                for ih in range(2):
                    s = spool.tile([128, 512], F32, tag="s", name="s")
                    for i in range(2):
                        nc.tensor.matmul(
                            s[:, 256 * i:256 * (i + 1)],
                            dr_ap(kt_, lo, 64, 128 * mt, 128, 0),
                            dr_ap(qt, lo, 64, 512 * ih + 256 * i, 256, 0),
                            start=True, stop=True, perf_mode=DR,
                            tile_position=(lo, 0))
                    halves.append(s)
                return halves

            def exp_par(halves, mt, par, ebuf):
                t, half = mt // 2, mt % 2
                et = ebuf[par][t]
                for ih in range(2):
                    dest = bass.AP(tensor=et.tensor,
                                   offset=et.offset + N * half + 512 * ih,
                                   ap=[[et.ap[0][0], 128], [1, 512]])
                    eng = exp_order[exp_i[0] % EXP_UNITS]
                    exp_i[0] += 1
                    if eng == "A":
                        nc.scalar.activation(out=dest, in_=halves[ih],
                                             func=EXP, scale=0.0625, bias=bkA)
                    else:
                        nc.vector.tensor_scalar(
                            out=dest.bitcast(U8), in0=halves[ih],
                            scalar1=SCH_S, scalar2=SCH_B, op0=MULT, op1=ADD)

            def pv_group(eh, vts, pair, g, aT):
                """PV + norm for chunks (2g, 2g+1): attnT bf16 into
                aT[:, 256g:256g+256]."""
                P = bank1.tile([128, 512], F32, tag="bank1", name="P")
                for ci in range(2):
                    c = 2 * g + ci
                    for h in range(2):
                        out = P[:, 130 * ci + 65 * h:130 * ci + 65 * (h + 1)]
                        for t in range(4):
                            nc.tensor.matmul(
                                out,
                                dr_ap(eh[h][t], 0, 128, 128 * c, 128, N),
                                dr_ap(vts[t], 0, 128, 65 * (2 * pair + h),
                                      65, 520),
                                start=(t == 0), stop=(t == 3), perf_mode=DR)
                rz = rzp.tile([128, 4], F32, tag="rz", name="rz")
                zc = bass.AP(tensor=P.tensor, offset=P.offset + 64,
                             ap=[[P.ap[0][0], 128], [65, 4]])
                nc.vector.reciprocal_approx_fast(out=rz, in_=zc)
                in0 = bass.AP(tensor=P.tensor, offset=P.offset,
                              ap=[[P.ap[0][0], 128], [65, 4], [1, HD]])
                in1 = bass.AP(tensor=rz.tensor, offset=rz.offset,
                              ap=[[rz.ap[0][0], 128], [1, 4], [0, HD]])
                nc.vector.tensor_mul(
                    out=aT[:, 256 * g:256 * (g + 1)], in0=in0, in1=in1)

            def transpose_half(aT, af, half):
                """attnT -> attn via xbar DMA transpose (DRAM bounce)."""
                tz = zdram.tile([128, 512], BF16, tag="tz", name="tz")
                nc.sync.dma_start(out=tz, in_=aT[:, 512 * half:512 * (half + 1)])
                nc.sync.dma_start_transpose(
                    out=af.rearrange("p (j n) -> p j n", j=MT)[:, 4 * half:4 * (half + 1), :],
                    in_=tz)

            def out_proj_chunk(xb, attn, b, ot, nch):
                """wo.T@attn; residual (with folded bias) added on VectorE."""
                ps = bank1.tile([128, 512], F32, tag="bank1", name="ps")
                for kc in range(KT):
                    nc.tensor.matmul(
                        ps,
                        wo[kc][:, 128 * ot:128 * (ot + 1)],
                        attn[kc][:, 512 * nch:512 * (nch + 1)],
                        start=(kc == 0), stop=(kc == KT - 1))
                osb = small.tile([128, 512], BF16, tag="osb")
                nc.vector.tensor_add(
                    out=osb, in0=ps,
                    in1=xb[:, N * ot + 512 * nch:N * ot + 512 * (nch + 1)])
                nc.sync.dma_start(
                    out=y[b, 128 * ot:128 * (ot + 1),
                          512 * nch:512 * (nch + 1)],
                    in_=osb)

            # ---- static state ----
            xf8_all = {0: xf8_t0}
            xb_all = {0: xb_t0}
            q_all, k_all, e_all, vT_all, aT_all, af_all = {}, {}, {}, {}, {}, {}

            def get_qk(b, pr):
                if (b, pr) not in q_all:
                    q_all[b, pr] = qkp.tile([128, N], F8E4, tag=f"q{pr}",
                                            name=f"q{pr}")
                    k_all[b, pr] = qkp.tile([128, N], F8E4, tag=f"k{pr}",
                                            name=f"k{pr}")
                return q_all[b, pr], k_all[b, pr]

            def mkjob(fn, *a):
                return lambda: fn(*a)

            def qk_jobs(b, pr):
                xf8 = xf8_all[b]
                jobs = []
                for nch in range(2):
                    for w in ("q", "k"):
                        q, k = get_qk(b, pr)
                        dest = q if w == "q" else k
                        jobs.append(mkjob(proj_qk_chunk, xf8, pr, w, nch, dest))
                return jobs

            def vt_jobs(b):
                vts = []
                for t in range(4):
                    vt = vtp.tile([128, 2, HEADS, HD + 1], F8E4,
                                  tag=f"vT{t}", name=f"vT{t}")
                    ones = bass.AP(tensor=vt.tensor, offset=vt.offset + HD,
                                   ap=[[vt.ap[0][0], 128], [65, 2 * HEADS],
                                       [1, 1]])
                    nc.gpsimd.memset(ones, 1.0)
                    vts.append(vt)
                vT_all[b] = vts
                xf8 = xf8_all[b]
                return [mkjob(proj_vT_mtile, xf8, vts[mt // 2], mt % 2, mt)
                        for mt in range(MT)]

            def op_jobs(b):
                return [mkjob(out_proj_chunk, xb_all[b], af_all[b], b,
                              ot, nch)
                        for nch in range(2) for ot in range(KT)]

            def prev_of(b, p):
                if (b, p) == (0, 0):
                    return None
                return (b, p - 1) if p > 0 else (b - 1, NPAIR - 1)

            def stretch(b, p, fillers):
                """Scores+exp for pair (b, p) interleaved with PV+norm of
                the previous pair and the filler jobs."""
                prev = prev_of(b, p)
                if prev is not None:
                    pb, pp = prev
                    aT = atp.tile([128, N], BF16, tag="aT", name="aT")
                    aT_all[prev] = aT
                e_all[b, p] = ebuf = [[], []]
                for par in range(2):
                    for t in range(4):
                        ebuf[par].append(epool.tile([128, 2, N], F8E4,
                                                    tag="e", name="e"))
                qt, kt_ = get_qk(b, p)
                nf = len(fillers)
                NS = 2 * MT
                # scores for tile t+1 are emitted BEFORE slot t's fillers so
                # a stalled filler never delays the score matmuls that the
                # exp engines are waiting on.
                s_cur = scores_par(qt, kt_, 0, 0)
                for sub in range(NS):
                    mt, par = sub // 2, sub % 2
                    if sub + 1 < NS:
                        s_nxt = scores_par(qt, kt_, (sub + 1) // 2,
                                           (sub + 1) % 2)
                    exp_par(s_cur, mt, par, ebuf)
                    if sub + 1 < NS:
                        s_cur = s_nxt
                    for i in range(nf * sub // NS, nf * (sub + 1) // NS):
                        fillers[i]()
                    if prev is not None and par == 1 and mt % 2 == 1:
                        pv_group(e_all[pb, pp], vT_all[pb], pp, mt // 2, aT)
                        if mt == 3 or mt == 7:
                            transpose_half(aT, af_all[pb][pp], mt // 4)

            def drain(b):
                """PV+norm+transpose for the final pair of batch b, then the
                batch's output projection, overlapped with the transposes."""
                pb, pp = b, NPAIR - 1
                aT = atp.tile([128, N], BF16, tag="aT", name="aT")
                aT_all[pb, pp] = aT
                ops = op_jobs(b)

                def pe_transpose_half(half):
                    tpf = bank1.tile([128, 512], F32, tag="bank1", name="tp")
                    tp = tpf.bitcast(BF16)
                    for ci in range(4):
                        c = 4 * half + ci
                        nc.tensor.transpose(
                            out=tp[:, 128 * ci:128 * (ci + 1)],
                            in_=aT[:, 128 * c:128 * (c + 1)],
                            identity=ident)
                        nc.scalar.activation(
                            out=af_all[pb][pp][:, 128 * c:128 * (c + 1)],
                            in_=tp[:, 128 * ci:128 * (ci + 1)],
                            func=COPY)

                pv_group(e_all[pb, pp], vT_all[pb], pp, 0, aT)
                pv_group(e_all[pb, pp], vT_all[pb], pp, 1, aT)
                pe_transpose_half(0)
                pv_group(e_all[pb, pp], vT_all[pb], pp, 2, aT)
                ops[0]()
                ops[1]()
                pv_group(e_all[pb, pp], vT_all[pb], pp, 3, aT)
                pe_transpose_half(1)
                ops[2]()
                ops[3]()
                for job in ops[4:]:
                    job()

            # ---- schedule ----
            for b in range(NB):
                af_all[b] = [afp.tile([128, N], BF16, tag=f"attn{t}",
                                      name=f"attn{t}") for t in range(KT)]

            qk00 = qk_jobs(0, 0)
            for j in qk00[0:3]:     # q-nch0, k-nch0, q-nch1; k-nch1 is not
                j()                 # needed until m-tile 4 -> first filler
            v0rest = qk00[3:] + vt_jobs(0)

            vt1_jobs = None
            op_jobs_b0 = None
            qk12_rest = None
            for b in range(NB):
                for p in range(NPAIR):
                    if b == 0 and p == 1:
                        xf8_all[1] = load_x8(1)
                        xb_all[1] = load_xb(1)
                        vt1_jobs = vt_jobs(1)
                    if b == 1 and p == 0:
                        op_jobs_b0 = op_jobs(0)
                    if b == 0 and p == 0:
                        f = v0rest + qk_jobs(0, 1)
                    elif b == 0 and p == 1:
                        f = qk_jobs(0, 2) + vt1_jobs[0:2]
                    elif b == 0 and p == 2:
                        f = qk_jobs(0, 3) + vt1_jobs[2:4]
                    elif b == 0 and p == 3:
                        f = qk_jobs(1, 0) + vt1_jobs[4:8]
                    elif b == 1 and p == 0:
                        qk12 = qk_jobs(1, 2)
                        f = qk_jobs(1, 1) + qk12[0:2]
                        qk12_rest = qk12[2:4]
                    elif b == 1 and p == 1:
                        f = qk12_rest + qk_jobs(1, 3) + op_jobs_b0[0:2]
                    elif b == 1 and p == 2:
                        f = op_jobs_b0[2:8]
                    else:
                        f = []
                    stretch(b, p, f)
            drain(NB - 1)

    nc.compile()
    return nc


def kernel(x, qkv_w, qkv_b, out_w, out_b, gamma):
    import ml_dtypes
    from concourse.bass_utils import run_bass_kernel_spmd

    x = np.asarray(x, dtype=np.float32)
    qkv_w = np.asarray(qkv_w, dtype=np.float32)
    qkv_b = np.asarray(qkv_b, dtype=np.float32)
    out_w = np.asarray(out_w, dtype=np.float32)
    out_b = np.asarray(out_b, dtype=np.float32)
    gamma = np.asarray(gamma, dtype=np.float32)

    if "nc" not in _cache:
        _cache["nc"] = _build_nc()
    nc = _cache["nc"]

    g = gamma.reshape(-1)[0]
    xf = x.reshape(B, C, N)
    # swizzle: row c_lo holds all KT contraction tiles side by side
    xs_sw = xf.reshape(B, KT, 128, N).transpose(0, 2, 1, 3).reshape(B, 128, KT * N)
    xs8 = np.ascontiguousarray(xs_sw.astype(ml_dtypes.float8_e4m3))
    # residual copy with the (gamma-scaled) output bias and the v-bias
    # contribution (softmax weights sum to 1) folded in
    xfb = xf + (g * out_b)[None, :, None]
    xsb = np.ascontiguousarray(
        xfb.reshape(B, KT, 128, N).transpose(0, 2, 1, 3)
        .reshape(B, 128, KT * N).astype(ml_dtypes.bfloat16))
    wT = qkv_w.T.reshape(KT, 128, 3 * C).transpose(1, 0, 2)  # (128, KT, 3C)
    qcols = wT[:, :, 0:C].reshape(128, KT, 4, 128)
    kcols = wT[:, :, C:2 * C].reshape(128, KT, 4, 128)
    qk_il = np.stack([qcols, kcols], axis=3)      # (128, KT, 4, 2, 128)
    wql = np.concatenate([qk_il.reshape(128, KT, 2 * C),
                          wT[:, :, 2 * C:3 * C]], axis=2)
    wqkvT = np.ascontiguousarray(
        wql.reshape(128, KT * 3 * C).astype(ml_dtypes.float8_e4m3))
    bq_col = np.ascontiguousarray(qkv_b.reshape(12, 128).T)  # (128, 12)
    bq_row = np.ascontiguousarray(qkv_b.reshape(1, 3 * C))
    woTg = np.ascontiguousarray((g * out_w).T.astype(ml_dtypes.bfloat16))
    identE = np.ascontiguousarray(np.eye(128, dtype=np.float32)
                                  .astype(ml_dtypes.bfloat16))

    in_maps = []
    for c in range(NCORES):
        in_maps.append({
            "xs8": np.ascontiguousarray(xs8[NB * c:NB * (c + 1)]),
            "xsb": np.ascontiguousarray(xsb[NB * c:NB * (c + 1)]),
            "wqkvT": wqkvT,
            "bqkv_col": bq_col,
            "bqkv_row": bq_row,
            "woTg": woTg,
            "identD": identE,
        })

    trace = bool(int(os.environ.get("KERNEL_TRACE", "0")))
    try:
        res = run_bass_kernel_spmd(nc, in_maps, core_ids=list(range(NCORES)),
                                   trace=trace)
    except ModuleNotFoundError:
        res = run_bass_kernel_spmd(nc, in_maps, core_ids=list(range(NCORES)),
                                   trace=False)
    _cache["last_result"] = res

    out = np.concatenate([np.asarray(res.results[c]["y"], dtype=np.float32)
                          for c in range(NCORES)], axis=0)
    return out.reshape(B, C, H, W)


# revision 53
# speedup vs baseline: 1.0212x; 1.0212x over previous
"""Trainium2 Bass kernel for ConvMHSA (B=16, C=512, H=W=32, 8 heads).

Data-parallel over batch: each of the 8 NeuronCores processes 2 batches.

v2: fp8(e4m3) DoubleRow matmuls throughout the attention path.
  - QKV projection: x and W in fp8; kc-pair DoubleRow blocks -> 4x fewer
    PE cycles than bf16. q/k written fp8 by ScalarE (Identity+bias), v
    written fp8 by VectorE (add broadcast bias), strided into per-mt-pair
    vT tiles [128, 2, 8, 65] with a persistent ones column (PV emits the
    softmax denominators Z for free).
  - Scores: q,k fp8; stride-0 DoubleRow blocks double the 64-contraction
    product (2*k.q, folded into the exp scale) at 0.5 cyc/row; two heads
    packed on the PE via tile_position row groups.
  - exp: global shift K (softmax-invariant) keeps e in e4m3 range.
    Work is split across three engines: ScalarE native Exp -> fp8 out;
    VectorE and Pool compute Schraudolph exp directly into fp8 bits via
    one tensor_scalar (f32 -> uint8 saturating convert, bitcast e4m3).
  - PV: e and vT fp8, m-tile-pair DoubleRow; two n-chunks share one PSUM
    bank ([c0h0|c0h1|c1h0|c1h1] x 65); one reciprocal [128,4] and one
    strided tensor_tensor mul per group normalizes into bf16 attnT.
  - attnT -> attn via the DMA-engine xbar transpose (DRAM bounce),
    PE transposes + ScalarE copies in the drain.
  - Output projection stays bf16 (xbar transpose requires 2-byte dtype);
    residual+bias via scalar_tensor_tensor on Pool; y staged bf16.

Emission software-pipelines at head-pair granularity as in v1: stretch
(b, p) emits pair p's scores/exp interleaved with pair p-1's PV/norm
plus filler projection jobs.
"""

import os
import sys

sys.path.insert(0, "/opt/trn_rl_repo")

import numpy as np

B, C, H, W = 16, 512, 32, 32
HEADS = 8
HD = C // HEADS          # 64
N = H * W                # 1024
NCORES = 8
NB = B // NCORES         # batches per core = 2
KT = C // 128            # 4 contraction tiles of 128
MT = N // 128            # 8 m-tiles / n-chunks
NPAIR = HEADS // 2       # 4 head-pairs

KSHIFT = 2.25            # global exp shift (softmax-invariant)
A8 = 8.0 / np.log(2.0)   # schraudolph e4m3 scale
SCH_S = A8 * 0.0625      # applied to doubled scores
SCH_B = 56.0 - 0.4375 - A8 * KSHIFT

# exp engine shares (of 256 half-tiles): ScalarE / VectorE. (Pool/GPSIMD
# cannot access PSUM on TRN2, so it only does memsets.)
EXP_SHARES = {"A": 139, "D": 117}
EXP_UNITS = 256

_cache = {}


def _exp_order():
    acc = {k: 0.0 for k in EXP_SHARES}
    order = []
    for _ in range(EXP_UNITS):
        for k in acc:
            acc[k] += EXP_SHARES[k] / EXP_UNITS
        pick = max(acc, key=lambda k: acc[k])
        acc[pick] -= 1.0
        order.append(pick)
    return order


def _build_nc():
    import concourse.bass as bass
    import concourse.tile as tile
    import concourse.mybir as mybir
    from concourse import bacc

    F32 = mybir.dt.float32
    F32R = mybir.dt.float32r
    BF16 = mybir.dt.bfloat16
    F8E4 = mybir.dt.float8e4
    U8 = mybir.dt.uint8
    EXP = mybir.ActivationFunctionType.Exp
    COPY = mybir.ActivationFunctionType.Copy
    IDENT = mybir.ActivationFunctionType.Identity
    DR = mybir.MatmulPerfMode.DoubleRow
    MULT = mybir.AluOpType.mult
    ADD = mybir.AluOpType.add

    nc = bacc.Bacc("TRN2", target_bir_lowering=False, debug=False,
                   num_devices=NCORES)

    xs8 = nc.dram_tensor("xs8", (NB, 128, KT * N), F8E4, kind="ExternalInput").ap()
    xsb = nc.dram_tensor("xsb", (NB, 128, KT * N), BF16, kind="ExternalInput").ap()
    wqkvT = nc.dram_tensor("wqkvT", (128, KT * 3 * C), F8E4, kind="ExternalInput").ap()
    bqkv_col = nc.dram_tensor("bqkv_col", (128, 12), F32, kind="ExternalInput").ap()
    bqkv_row = nc.dram_tensor("bqkv_row", (1, 3 * C), F32R, kind="ExternalInput").ap()
    woTg = nc.dram_tensor("woTg", (C, C), BF16, kind="ExternalInput").ap()

    identD = nc.dram_tensor("identD", (128, 128), BF16, kind="ExternalInput").ap()
    y = nc.dram_tensor("y", (NB, C, N), BF16, kind="ExternalOutput").ap()

    exp_order = _exp_order()
    exp_i = [0]

    with tile.TileContext(nc) as tc:
        with tc.tile_pool(name="const", bufs=1) as const, \
             tc.tile_pool(name="xfp", bufs=2) as xfp, \
             tc.tile_pool(name="xbp", bufs=2) as xbp, \
             tc.tile_pool(name="qkp", bufs=1) as qkp, \
             tc.tile_pool(name="epool", bufs=20) as epool, \
             tc.tile_pool(name="vtp", bufs=2) as vtp, \
             tc.tile_pool(name="atp", bufs=3) as atp, \
             tc.tile_pool(name="afp", bufs=2) as afp, \
             tc.tile_pool(name="small", bufs=4) as small, \
             tc.tile_pool(name="rzp", bufs=8) as rzp, \
             tc.tile_pool(name="spool", bufs=4, space="PSUM") as spool, \
             tc.tile_pool(name="bank1", bufs=4, space="PSUM") as bank1, \
             tc.tile_pool(name="zdram", bufs=6, space="DRAM") as zdram:

            # ---- weights/x: big strided DMAs, first-needed columns first,
            # spread across the three HWDGE queues (SP/ACT/DVE idle here) ----
            wq_t = const.tile([128, KT * 3 * C], F8E4, tag="wq", name="wq_t")
            wq3 = wq_t.rearrange("p (k o) -> p k o", k=KT)
            wsrc = wqkvT.rearrange("p (k o) -> p k o", k=KT)
            # per-kc column layout: [q_p0 k_p0 q_p1 k_p1 ... | v]
            nc.sync.dma_start(out=wq3[:, :, 0:256], in_=wsrc[:, :, 0:256])

            xf8_t0 = xfp.tile([128, KT * N], F8E4, tag="xf8", name="xf8_t")
            x83 = xf8_t0.rearrange("p (k n) -> p k n", k=KT)
            x0src = xs8[0].rearrange("p (k n) -> p k n", k=KT)
            nc.sync.dma_start(out=x83[:, :, 0:512], in_=x0src[:, :, 0:512])
            bq_col = const.tile([128, 12], F32, tag="bqcol")
            nc.sync.dma_start(out=bq_col, in_=bqkv_col)
            nc.sync.dma_start(out=x83[:, :, 512:N], in_=x0src[:, :, 512:N])
            # v columns, then the remaining q/k columns
            nc.sync.dma_start(out=wq3[:, :, 1024:1536], in_=wsrc[:, :, 1024:1536])
            ident = const.tile([128, 128], BF16, tag="ident", name="ident")
            nc.sync.dma_start(out=ident, in_=identD)
            nc.sync.dma_start(out=wq3[:, :, 256:1024], in_=wsrc[:, :, 256:1024])
            wo = []
            for kc in range(KT):
                t = const.tile([128, C], BF16, tag=f"wo{kc}", name=f"wo{kc}")
                nc.sync.dma_start(out=t, in_=woTg[128 * kc:128 * (kc + 1), :])
                wo.append(t)

            bv_bc = const.tile([128, C], F32, tag="bvbc")
            bv_src = bass.AP(tensor=bqkv_row.tensor, offset=2 * C,
                             ap=[[0, 128], [1, C]])
            nc.sync.dma_start(out=bv_bc, in_=bv_src.bitcast(F32))
            bkA = const.tile([128, 1], F32, tag="bkA")
            nc.gpsimd.memset(bkA, -KSHIFT)

            # bf16 x for the output-projection residual
            xb_t0 = xbp.tile([128, KT * N], BF16, tag="xb", name="xb_t")
            nc.sync.dma_start(out=xb_t0, in_=xsb[0])

            # PE p-state warm-up (results never read)
            warm = const.tile([128, 512], BF16, tag="warm", name="warm")
            nc.gpsimd.memset(warm, 1.0)
            for i in range(9):
                wps = bank1.tile([128, 512], F32, tag="bank1", name="wps")
                nc.tensor.matmul(wps, warm[:, 0:128], warm,
                                 start=True, stop=True)

            def load_x8(b):
                t = xfp.tile([128, KT * N], F8E4, tag="xf8", name="xf8_t")
                t3 = t.rearrange("p (k n) -> p k n", k=KT)
                src = xs8[b].rearrange("p (k n) -> p k n", k=KT)
                nc.sync.dma_start(out=t3[:, :, 0:512], in_=src[:, :, 0:512])
                nc.sync.dma_start(out=t3[:, :, 512:N], in_=src[:, :, 512:N])
                return t

            def load_xb(b):
                t = xbp.tile([128, KT * N], BF16, tag="xb", name="xb_t")
                nc.sync.dma_start(out=t, in_=xsb[b])
                return t

            def dr_ap(t, row0, nrows, col0, ncols, blk_stride):
                """[nrows, 2, ncols] AP with DoubleRow blocks at blk_stride."""
                return bass.AP(tensor=t.tensor,
                               offset=t.offset + row0 * t.ap[0][0] + col0,
                               ap=[[t.ap[0][0], nrows], [blk_stride, 2],
                                   [1, ncols]])

            # ---- building blocks ----
            def proj_qk_chunk(xf8, pair, which, nch, dest):
                """One (q|k, nch) 512-col chunk of the pair projection."""
                ot = pair if which == "q" else KT + pair
                col = 256 * pair + (0 if which == "q" else 128)
                ps = bank1.tile([128, 512], F32, tag="bank1", name="ps")
                for i in range(2):
                    for j in range(2):
                        nc.tensor.matmul(
                            ps[:, 256 * i:256 * (i + 1)],
                            dr_ap(wq_t, 0, 128, 1536 * 2 * j + col, 128, 1536),
                            dr_ap(xf8, 0, 128,
                                  1024 * 2 * j + 512 * nch + 256 * i, 256, 1024),
                            start=(j == 0), stop=(j == 1), perf_mode=DR)
                nc.scalar.activation(
                    out=dest[:, 512 * nch:512 * (nch + 1)], in_=ps,
                    func=IDENT, bias=bq_col[:, ot:ot + 1])

            def proj_vT_mtile(xf8, vt, half, mt):
                """v m-tile -> half of a vT pair tile [128, 2, 8, 65] fp8."""
                ps = bank1.tile([128, 512], F32, tag="bank1", name="ps")
                for i in range(2):
                    for j in range(2):
                        nc.tensor.matmul(
                            ps[:, 256 * i:256 * (i + 1)],
                            dr_ap(xf8, 0, 128, 1024 * 2 * j + 128 * mt, 128, 1024),
                            dr_ap(wq_t, 0, 128,
                                  1536 * 2 * j + 1024 + 256 * i, 256, 1536),
                            start=(j == 0), stop=(j == 1), perf_mode=DR)
                # v bias is folded into the residual on the host (softmax
                # weights sum to 1), so this is a plain convert-copy.
                dest = bass.AP(tensor=vt.tensor,
                               offset=vt.offset + 520 * half,
                               ap=[[vt.ap[0][0], 128], [65, HEADS], [1, HD]])
                nc.vector.tensor_add(
                    out=dest,
                    in0=ps.rearrange("p (a b) -> p a b", a=HEADS),
                    in1=bv_bc.rearrange("p (a b) -> p a b", a=HEADS))

            def scores_par(qt, kt_, mt, par):
                """Scores for one head of the pair at m-tile mt: two
                single-bank PSUM halves so the exp pipeline is 4 deep."""
                lo = 64 * par
                halves = []
                for ih in range(2):
                    s = spool.tile([128, 512], F32, tag="s", name="s")
                    for i in range(2):
                        nc.tensor.matmul(
                            s[:, 256 * i:256 * (i + 1)],
                            dr_ap(kt_, lo, 64, 128 * mt, 128, 0),
                            dr_ap(qt, lo, 64, 512 * ih + 256 * i, 256, 0),
                            start=True, stop=True, perf_mode=DR,
                            tile_position=(lo, 0))
                    halves.append(s)
                return halves

            def exp_par(halves, mt, par, ebuf):
                t, half = mt // 2, mt % 2
                et = ebuf[par][t]
                for ih in range(2):
                    dest = bass.AP(tensor=et.tensor,
                                   offset=et.offset + N * half + 512 * ih,
                                   ap=[[et.ap[0][0], 128], [1, 512]])
                    eng = exp_order[exp_i[0] % EXP_UNITS]
                    exp_i[0] += 1
                    if eng == "A":
                        nc.scalar.activation(out=dest, in_=halves[ih],
                                             func=EXP, scale=0.0625, bias=bkA)
                    else:
                        nc.vector.tensor_scalar(
                            out=dest.bitcast(U8), in0=halves[ih],
                            scalar1=SCH_S, scalar2=SCH_B, op0=MULT, op1=ADD)

            def pv_group(eh, vts, pair, g, aT):
                """PV + norm for chunks (2g, 2g+1): attnT bf16 into
                aT[:, 256g:256g+256]."""
                P = bank1.tile([128, 512], F32, tag="bank1", name="P")
                for ci in range(2):
                    c = 2 * g + ci
                    for h in range(2):
                        out = P[:, 130 * ci + 65 * h:130 * ci + 65 * (h + 1)]
                        for t in range(4):
                            nc.tensor.matmul(
                                out,
                                dr_ap(eh[h][t], 0, 128, 128 * c, 128, N),
                                dr_ap(vts[t], 0, 128, 65 * (2 * pair + h),
                                      65, 520),
                                start=(t == 0), stop=(t == 3), perf_mode=DR)
                rz = rzp.tile([128, 4], F32, tag="rz", name="rz")
                zc = bass.AP(tensor=P.tensor, offset=P.offset + 64,
                             ap=[[P.ap[0][0], 128], [65, 4]])
                nc.vector.reciprocal_approx_fast(out=rz, in_=zc)
                in0 = bass.AP(tensor=P.tensor, offset=P.offset,
                              ap=[[P.ap[0][0], 128], [65, 4], [1, HD]])
                in1 = bass.AP(tensor=rz.tensor, offset=rz.offset,
                              ap=[[rz.ap[0][0], 128], [1, 4], [0, HD]])
                nc.vector.tensor_mul(
                    out=aT[:, 256 * g:256 * (g + 1)], in0=in0, in1=in1)

            def transpose_half(aT, af, half):
                """attnT -> attn via xbar DMA transpose (DRAM bounce)."""
                tz = zdram.tile([128, 512], BF16, tag="tz", name="tz")
                nc.sync.dma_start(out=tz, in_=aT[:, 512 * half:512 * (half + 1)])
                nc.sync.dma_start_transpose(
                    out=af.rearrange("p (j n) -> p j n", j=MT)[:, 4 * half:4 * (half + 1), :],
                    in_=tz)

            def out_proj_chunk(xb, attn, b, ot, nch):
                """wo.T@attn; residual (with folded bias) added on VectorE."""
                ps = bank1.tile([128, 512], F32, tag="bank1", name="ps")
                for kc in range(KT):
                    nc.tensor.matmul(
                        ps,
                        wo[kc][:, 128 * ot:128 * (ot + 1)],
                        attn[kc][:, 512 * nch:512 * (nch + 1)],
                        start=(kc == 0), stop=(kc == KT - 1))
                osb = small.tile([128, 512], BF16, tag="osb")
                nc.vector.tensor_add(
                    out=osb, in0=ps,
                    in1=xb[:, N * ot + 512 * nch:N * ot + 512 * (nch + 1)])
                nc.sync.dma_start(
                    out=y[b, 128 * ot:128 * (ot + 1),
                          512 * nch:512 * (nch + 1)],
                    in_=osb)

            # ---- static state ----
            xf8_all = {0: xf8_t0}
            xb_all = {0: xb_t0}
            q_all, k_all, e_all, vT_all, aT_all, af_all = {}, {}, {}, {}, {}, {}

            def get_qk(b, pr):
                if (b, pr) not in q_all:
                    q_all[b, pr] = qkp.tile([128, N], F8E4, tag=f"q{pr}",
                                            name=f"q{pr}")
                    k_all[b, pr] = qkp.tile([128, N], F8E4, tag=f"k{pr}",
                                            name=f"k{pr}")
                return q_all[b, pr], k_all[b, pr]

            def mkjob(fn, *a):
                return lambda: fn(*a)

            def qk_jobs(b, pr):
                xf8 = xf8_all[b]
                jobs = []
                for nch in range(2):
                    for w in ("q", "k"):
                        q, k = get_qk(b, pr)
                        dest = q if w == "q" else k
                        jobs.append(mkjob(proj_qk_chunk, xf8, pr, w, nch, dest))
                return jobs

            def vt_jobs(b):
                vts = []
                for t in range(4):
                    vt = vtp.tile([128, 2, HEADS, HD + 1], F8E4,
                                  tag=f"vT{t}", name=f"vT{t}")
                    ones = bass.AP(tensor=vt.tensor, offset=vt.offset + HD,
                                   ap=[[vt.ap[0][0], 128], [65, 2 * HEADS],
                                       [1, 1]])
                    nc.gpsimd.memset(ones, 1.0)
                    vts.append(vt)
                vT_all[b] = vts
                xf8 = xf8_all[b]
                return [mkjob(proj_vT_mtile, xf8, vts[mt // 2], mt % 2, mt)
                        for mt in range(MT)]

            def op_jobs(b):
                return [mkjob(out_proj_chunk, xb_all[b], af_all[b], b,
                              ot, nch)
                        for nch in range(2) for ot in range(KT)]

            def prev_of(b, p):
                if (b, p) == (0, 0):
                    return None
                return (b, p - 1) if p > 0 else (b - 1, NPAIR - 1)

            def stretch(b, p, fillers):
                """Scores+exp for pair (b, p) interleaved with PV+norm of
                the previous pair and the filler jobs."""
                prev = prev_of(b, p)
                if prev is not None:
                    pb, pp = prev
                    aT = atp.tile([128, N], BF16, tag="aT", name="aT")
                    aT_all[prev] = aT
                e_all[b, p] = ebuf = [[], []]
                for par in range(2):
                    for t in range(4):
                        ebuf[par].append(epool.tile([128, 2, N], F8E4,
                                                    tag="e", name="e"))
                qt, kt_ = get_qk(b, p)
                nf = len(fillers)
                NS = 2 * MT
                # scores for tile t+1 are emitted BEFORE slot t's fillers so
                # a stalled filler never delays the score matmuls that the
                # exp engines are waiting on.
                s_cur = scores_par(qt, kt_, 0, 0)
                for sub in range(NS):
                    mt, par = sub // 2, sub % 2
                    if sub + 1 < NS:
                        s_nxt = scores_par(qt, kt_, (sub + 1) // 2,
                                           (sub + 1) % 2)
                    exp_par(s_cur, mt, par, ebuf)
                    if sub + 1 < NS:
                        s_cur = s_nxt
                    for i in range(nf * sub // NS, nf * (sub + 1) // NS):
                        fillers[i]()
                    if prev is not None and par == 1 and mt % 2 == 1:
                        pv_group(e_all[pb, pp], vT_all[pb], pp, mt // 2, aT)
                        if mt == 3 or mt == 7:
                            transpose_half(aT, af_all[pb][pp], mt // 4)

            def drain(b):
                """PV+norm+transpose for the final pair of batch b, then the
                batch's output projection, overlapped with the transposes."""
                pb, pp = b, NPAIR - 1
                aT = atp.tile([128, N], BF16, tag="aT", name="aT")
                aT_all[pb, pp] = aT
                ops = op_jobs(b)

                def pe_transpose_half(half):
                    tpf = bank1.tile([128, 512], F32, tag="bank1", name="tp")
                    tp = tpf.bitcast(BF16)
                    for ci in range(4):
                        c = 4 * half + ci
                        nc.tensor.transpose(
                            out=tp[:, 128 * ci:128 * (ci + 1)],
                            in_=aT[:, 128 * c:128 * (c + 1)],
                            identity=ident)
                        nc.scalar.activation(
                            out=af_all[pb][pp][:, 128 * c:128 * (c + 1)],
                            in_=tp[:, 128 * ci:128 * (ci + 1)],
                            func=COPY)

                pv_group(e_all[pb, pp], vT_all[pb], pp, 0, aT)
                pv_group(e_all[pb, pp], vT_all[pb], pp, 1, aT)
                pe_transpose_half(0)
                pv_group(e_all[pb, pp], vT_all[pb], pp, 2, aT)
                ops[0]()
                ops[1]()
                pv_group(e_all[pb, pp], vT_all[pb], pp, 3, aT)
                pe_transpose_half(1)
                ops[2]()
                ops[3]()
                for job in ops[4:]:
                    job()

            # ---- schedule ----
            for b in range(NB):
                af_all[b] = [afp.tile([128, N], BF16, tag=f"attn{t}",
                                      name=f"attn{t}") for t in range(KT)]

            qk00 = qk_jobs(0, 0)
            for j in qk00[0:3]:     # q-nch0, k-nch0, q-nch1; k-nch1 is not
                j()                 # needed until m-tile 4 -> first filler
            v0rest = qk00[3:] + vt_jobs(0)

            vt1_jobs = None
            op_jobs_b0 = None
            qk12_rest = None
            for b in range(NB):
                for p in range(NPAIR):
                    if b == 0 and p == 1:
                        xf8_all[1] = load_x8(1)
                        xb_all[1] = load_xb(1)
                        vt1_jobs = vt_jobs(1)
                    if b == 1 and p == 0:
                        op_jobs_b0 = op_jobs(0)
                    if b == 0 and p == 0:
                        f = v0rest + qk_jobs(0, 1)
                    elif b == 0 and p == 1:
                        f = qk_jobs(0, 2) + vt1_jobs[0:2]
                    elif b == 0 and p == 2:
                        f = qk_jobs(0, 3) + vt1_jobs[2:4]
                    elif b == 0 and p == 3:
                        f = qk_jobs(1, 0) + vt1_jobs[4:8]
                    elif b == 1 and p == 0:
                        qk12 = qk_jobs(1, 2)
                        f = qk_jobs(1, 1) + qk12[0:2]
                        qk12_rest = qk12[2:4]
                    elif b == 1 and p == 1:
                        f = qk12_rest + qk_jobs(1, 3) + op_jobs_b0[0:2]
                    elif b == 1 and p == 2:
                        f = op_jobs_b0[2:8]
                    else:
                        f = []
                    stretch(b, p, f)
            drain(NB - 1)

    nc.compile()
    return nc


def kernel(x, qkv_w, qkv_b, out_w, out_b, gamma):
    import ml_dtypes
    from concourse.bass_utils import run_bass_kernel_spmd

    x = np.asarray(x, dtype=np.float32)
    qkv_w = np.asarray(qkv_w, dtype=np.float32)
    qkv_b = np.asarray(qkv_b, dtype=np.float32)
    out_w = np.asarray(out_w, dtype=np.float32)
    out_b = np.asarray(out_b, dtype=np.float32)
    gamma = np.asarray(gamma, dtype=np.float32)

    if "nc" not in _cache:
        _cache["nc"] = _build_nc()
    nc = _cache["nc"]

    g = gamma.reshape(-1)[0]
    xf = x.reshape(B, C, N)
    # swizzle: row c_lo holds all KT contraction tiles side by side
    xs_sw = xf.reshape(B, KT, 128, N).transpose(0, 2, 1, 3).reshape(B, 128, KT * N)
    xs8 = np.ascontiguousarray(xs_sw.astype(ml_dtypes.float8_e4m3))
    # residual copy with the (gamma-scaled) output bias and the v-bias
    # contribution (softmax weights sum to 1) folded in
    xfb = xf + (g * out_b)[None, :, None]
    xsb = np.ascontiguousarray(
        xfb.reshape(B, KT, 128, N).transpose(0, 2, 1, 3)
        .reshape(B, 128, KT * N).astype(ml_dtypes.bfloat16))
    wT = qkv_w.T.reshape(KT, 128, 3 * C).transpose(1, 0, 2)  # (128, KT, 3C)
    qcols = wT[:, :, 0:C].reshape(128, KT, 4, 128)
    kcols = wT[:, :, C:2 * C].reshape(128, KT, 4, 128)
    qk_il = np.stack([qcols, kcols], axis=3)      # (128, KT, 4, 2, 128)
    wql = np.concatenate([qk_il.reshape(128, KT, 2 * C),
                          wT[:, :, 2 * C:3 * C]], axis=2)
    wqkvT = np.ascontiguousarray(
        wql.reshape(128, KT * 3 * C).astype(ml_dtypes.float8_e4m3))
    bq_col = np.ascontiguousarray(qkv_b.reshape(12, 128).T)  # (128, 12)
    bq_row = np.ascontiguousarray(qkv_b.reshape(1, 3 * C))
    woTg = np.ascontiguousarray((g * out_w).T.astype(ml_dtypes.bfloat16))
    identE = np.ascontiguousarray(np.eye(128, dtype=np.float32)
                                  .astype(ml_dtypes.bfloat16))

    in_maps = []
    for c in range(NCORES):
        in_maps.append({
            "xs8": np.ascontiguousarray(xs8[NB * c:NB * (c + 1)]),
            "xsb": np.ascontiguousarray(xsb[NB * c:NB * (c + 1)]),
            "wqkvT": wqkvT,
            "bqkv_col": bq_col,
            "bqkv_row": bq_row,
            "woTg": woTg,
            "identD": identE,
        })

    trace = bool(int(os.environ.get("KERNEL_TRACE", "0")))
    try:
        res = run_bass_kernel_spmd(nc, in_maps, core_ids=list(range(NCORES)),
                                   trace=trace)
    except ModuleNotFoundError:
        res = run_bass_kernel_spmd(nc, in_maps, core_ids=list(range(NCORES)),
                                   trace=False)
    _cache["last_result"] = res

    out = np.concatenate([np.asarray(res.results[c]["y"], dtype=np.float32)
                          for c in range(NCORES)], axis=0)
    return out.reshape(B, C, H, W)
